# revision 5
# baseline (speedup 1.0000x reference)
"""Pre-LN transformer block (B=2,T=2048,C=1024,H=16) on 8 TRN2 NeuronCores.

Two SPMD launches:
  L1: tensor-parallel over heads (2 heads/core). LN1 stats computed on-chip
      via ones-matmul (born broadcast), rstd via Sqrt+DVE reciprocal (no ACT
      table thrash), LN folded algebraically into QKV output corrections.
      kt-major causal attention with on-chip softmax; batch-1 QKV work is
      interleaved with batch-0 attention to keep the PE dense.
  L2: parallel over token rows (512 rows/core) - output projection +
      residual, LN2, FFN (relu) + residual. w2 is fully SBUF-resident
      (loaded into the region freed by at/wp/xr), w1 streamed in 2MB waves.
Host work between launches is just resharding. All matmuls run in bf16 with
fp32 PSUM accumulation; residuals are carried in fp32.
"""
import contextlib
import numpy as np
import ml_dtypes

import concourse.bass as bass
import concourse.mybir as mybir
import concourse.tile as tile
from concourse import bacc
from concourse.masks import make_identity

bf16 = ml_dtypes.bfloat16
FP32 = mybir.dt.float32
BF16 = mybir.dt.bfloat16
AF = mybir.ActivationFunctionType

B, T, C, H = 2, 2048, 1024, 16
HS = C // H          # 64
NCORES = 8
HPC = H // NCORES    # 2 heads per core
TOK = B * T          # 4096
EPS = 1e-5
CT = C // 128        # 8 c-tiles
NCH = TOK // 512     # 8 512-col chunks of token axis
QB = 512             # query block
ROWS = TOK // NCORES # 512 rows per core in L2
HID = 4 * C          # 4096
HT = HID // 128      # 32 hidden tiles
MT = ROWS // 128     # 4 token tiles in L2
KT_B = T // 128      # 16 key tiles per batch


def build_l1(use_beta=True):
    nc = bacc.Bacc("TRN2", target_bir_lowering=False, debug=False, num_devices=NCORES)
    xt_d = nc.dram_tensor("xt", [C, TOK], BF16, kind="ExternalInput").ap()
    wq_d = nc.dram_tensor("wq", [C, 128], BF16, kind="ExternalInput").ap()
    wk_d = nc.dram_tensor("wk", [C, 128], BF16, kind="ExternalInput").ap()
    wv_d = nc.dram_tensor("wv", [C, 128], BF16, kind="ExternalInput").ap()
    # negated column sums of wq/wk/wv and W.T @ beta1, all [128,3] fp32
    nws_d = nc.dram_tensor("nws", [128, 3], FP32, kind="ExternalInput").ap()
    wb_d = nc.dram_tensor("wb", [128, 3], FP32, kind="ExternalInput").ap()
    tri_d = nc.dram_tensor("tri", [128, 128], BF16, kind="ExternalInput").ap()
    out_d = nc.dram_tensor("attn_out", [128, TOK], BF16, kind="ExternalOutput").ap()

    xtr = xt_d.rearrange("(a p) m -> p a m", p=128)

    with tile.TileContext(nc) as tc, contextlib.ExitStack() as ctx:
        consts = ctx.enter_context(tc.tile_pool(name="consts", bufs=1))
        hpool = ctx.enter_context(tc.tile_pool(name="hT", bufs=1))
        stats = ctx.enter_context(tc.tile_pool(name="stats", bufs=1))
        tmp = ctx.enter_context(tc.tile_pool(name="tmp", bufs=2))
        qkv = ctx.enter_context(tc.tile_pool(name="qkv", bufs=1))
        probs_p = ctx.enter_context(tc.tile_pool(name="probs", bufs=10))
        flush_p = ctx.enter_context(tc.tile_pool(name="flush", bufs=3))
        ps_rot = ctx.enter_context(tc.tile_pool(name="ps_rot", bufs=4, space="PSUM"))
        ps_acc = ctx.enter_context(tc.tile_pool(name="ps_acc", bufs=4, space="PSUM"))

        # ---- constants ----
        ones_sb = consts.tile([128, 128], BF16)
        nc.vector.memset(ones_sb, 1.0 / C)   # fold 1/C into the stats matmul
        onesw = consts.tile([128, 128], BF16)
        nc.vector.memset(onesw, 1.0)
        eps_sb = consts.tile([128, 1], FP32)
        nc.vector.memset(eps_sb, EPS)
        ident = consts.tile([128, 128], BF16)
        make_identity(nc, ident)

        # PE warm-up spin: keep HAM busy while input DMAs stream
        warm_ps = ps_rot.tile([128, 512], FP32, tag="mm")
        for _ in range(30):
            nc.tensor.matmul(warm_ps[:, 0:128], onesw, onesw, start=True, stop=True)

        # ---- bulk input DMAs: few big transfers, multiple HWDGE rings ----
        wq_sb = consts.tile([128, CT, 128], BF16)
        nc.scalar.dma_start(out=wq_sb, in_=wq_d.rearrange("(a p) m -> p a m", p=128))
        wk_sb = consts.tile([128, CT, 128], BF16)
        nc.scalar.dma_start(out=wk_sb, in_=wk_d.rearrange("(a p) m -> p a m", p=128))
        wv_sb = consts.tile([128, CT, 128], BF16)
        nc.scalar.dma_start(out=wv_sb, in_=wv_d.rearrange("(a p) m -> p a m", p=128))
        nws_sb = consts.tile([128, 3], FP32)
        nc.gpsimd.dma_start(out=nws_sb, in_=nws_d)
        wb_sb = consts.tile([128, 3], FP32)
        nc.gpsimd.dma_start(out=wb_sb, in_=wb_d)
        tri_sb = consts.tile([128, 128], BF16)
        nc.gpsimd.dma_start(out=tri_sb, in_=tri_d)

        # xT as one [128, CT, TOK] tile, loaded chunk-major in 8 x 1MB DMAs
        xts = hpool.tile([128, CT, TOK], BF16, tag="xt_sb")
        for j in range(NCH):
            sl = slice(j * 512, (j + 1) * 512)
            nc.sync.dma_start(out=xts[:, :, sl], in_=xtr[:, :, sl])

        rstd_b = stats.tile([128, TOK], BF16, tag="rstd_b")
        murstd_b = stats.tile([128, TOK], BF16, tag="murstd_b")
        qt_sb = qkv.tile([128, TOK], BF16, tag="qt")
        kt_sb = qkv.tile([128, TOK], BF16, tag="kt")
        vt_sb = qkv.tile([128, TOK], BF16, tag="vt")
        # V' tiles: [v_h0 | ones | v_h1 | ones] per key tile; memset once so
        # the two ones-columns (64, 129) never need individual memsets
        vts = qkv.tile([128, TOK // 128, 130], BF16, tag="vts")
        nc.vector.memset(vts, 1.0)

        def chunk_steps(j):
            """LN1 stats + QKV for one 512-token chunk (generator: yields
            between PE-op groups for interleaving)."""
            sl = slice(j * 512, (j + 1) * 512)
            # stats matmuls: ones(1/C) stationary -> every PSUM partition
            # carries the same scaled column sums (stats born broadcast)
            ps_sum = ps_rot.tile([128, 512], FP32, tag="mm")
            ps_sq = ps_rot.tile([128, 512], FP32, tag="mm")
            for ci in range(CT):
                sq = tmp.tile([128, 512], BF16, tag="sq")
                nc.vector.tensor_mul(sq, xts[:, ci, sl], xts[:, ci, sl])
                nc.tensor.matmul(ps_sum, ones_sb, xts[:, ci, sl],
                                 start=(ci == 0), stop=(ci == CT - 1))
                nc.tensor.matmul(ps_sq, ones_sb, sq,
                                 start=(ci == 0), stop=(ci == CT - 1))
            # rstd = 1/sqrt(var+eps): Square/Sqrt live in every ACT table
            # (no table reloads); reciprocal on DVE
            mu2 = tmp.tile([128, 512], FP32, tag="mu2")
            nc.scalar.activation(out=mu2, in_=ps_sum, func=AF.Square, scale=1.0)
            varr = tmp.tile([128, 512], FP32, tag="varr")
            nc.vector.tensor_sub(varr, ps_sq, mu2)
            srt = tmp.tile([128, 512], FP32, tag="srt")
            nc.scalar.activation(out=srt, in_=varr, func=AF.Sqrt,
                                 bias=eps_sb, scale=1.0)
            with nc.allow_low_precision("rstd kept in bf16, as baseline"):
                nc.vector.reciprocal(rstd_b[:, sl], srt)
            nc.vector.tensor_mul(murstd_b[:, sl], ps_sum, rstd_b[:, sl])
            yield
            # QKV on RAW xT; LN folded in afterwards:
            #   QT = rstd*(Wq.T@xT) + (-colsum(Wq))*murstd [+ Wq.T@beta]
            for wsb, idx, tsb in ((wq_sb, 0, qt_sb), (wk_sb, 1, kt_sb),
                                  (wv_sb, 2, vt_sb)):
                ps = ps_rot.tile([128, 512], FP32, tag="mm")
                for ci in range(CT):
                    nc.tensor.matmul(ps, wsb[:, ci, :], xts[:, ci, sl],
                                     start=(ci == 0), stop=(ci == CT - 1))
                nc.scalar.copy(out=tsb[:, sl], in_=ps)
                nc.vector.tensor_mul(tsb[:, sl], tsb[:, sl], rstd_b[:, sl])
                nc.vector.scalar_tensor_tensor(
                    tsb[:, sl], murstd_b[:, sl], nws_sb[:, idx:idx + 1],
                    tsb[:, sl], op0=mybir.AluOpType.mult,
                    op1=mybir.AluOpType.add)
                if use_beta:
                    nc.vector.tensor_scalar_add(tsb[:, sl], tsb[:, sl],
                                                wb_sb[:, idx:idx + 1])
                yield

        def vt_block(b):
            """PE-transpose V for one batch into vts (DVE drains)."""
            for tt in range(b * KT_B, (b + 1) * KT_B):
                ptv = ps_rot.tile([128, 128], BF16, tag="mm")
                nc.tensor.transpose(ptv, vt_sb[:, tt * 128:(tt + 1) * 128], ident)
                nc.vector.tensor_scalar_mul(
                    vts[:, tt, :].rearrange("p (g c) -> p g c", g=2)[:, :, 0:64],
                    ptv.rearrange("p (g c) -> p g c", g=2), 1.0)
            yield

        # ---- kt-major causal attention for one (batch, head) group ----
        scale = C ** -0.5

        def attn_group(b, hl):
            hsl = slice(hl * 64, (hl + 1) * 64)
            vcol = slice(hl * 65, hl * 65 + 65)
            pa = [None] * 4
            pend = []  # attnV ops for the previous kt (1-deep sw pipeline)

            def emit_attnv(items):
                for (kt, j, pr, c0) in items:
                    if pa[j] is None:
                        pa[j] = ps_acc.tile([65, 512], FP32, tag="pa",
                                            name=f"pa{b}{hl}{j}")
                    nc.tensor.matmul(pa[j][:, c0:] if c0 else pa[j],
                                     vts[:, b * KT_B + kt, vcol],
                                     pr[:, c0:] if c0 else pr,
                                     start=(kt == 0), stop=(kt == 4 * j + 3),
                                     skip_group_check=True)
                    if kt == 4 * j + 3:
                        # flush pa[j]: denominator row -> broadcast ->
                        # reciprocal; numerator scaled straight out of PSUM
                        den = flush_p.tile([1, 512], FP32, tag="den")
                        nc.scalar.copy(out=den, in_=pa[j][64:65, :])
                        rec = flush_p.tile([64, 512], FP32, tag="rec")
                        nc.gpsimd.partition_broadcast(rec, den, channels=64)
                        recf = flush_p.tile([64, 512], FP32, tag="recf")
                        nc.vector.reciprocal_approx_fast(recf, rec)
                        ao = flush_p.tile([64, 512], BF16, tag="ao")
                        nc.vector.tensor_mul(ao, pa[j][0:64, :], recf)
                        q0 = b * T + j * QB
                        nc.gpsimd.dma_start(
                            out=out_d[hl * 64:(hl + 1) * 64, q0:q0 + QB], in_=ao)
                        pa[j] = None

            for kt in range(KT_B):
                koff = b * T + kt * 128
                cur = []
                for j in range(kt // 4, 4):
                    d = kt - 4 * j
                    c0 = 128 * d if d > 0 else 0
                    q0 = b * T + j * QB
                    ps = ps_rot.tile([128, 512], FP32, tag="mm")
                    nc.tensor.matmul(ps[:, c0:], kt_sb[hsl, koff:koff + 128],
                                     qt_sb[hsl, q0 + c0:q0 + QB],
                                     start=True, stop=True)
                    pr = probs_p.tile([128, 512], BF16, tag="pr")
                    nc.scalar.activation(out=pr[:, c0:], in_=ps[:, c0:],
                                         func=AF.Exp, scale=scale)
                    if d >= 0:
                        nc.vector.tensor_mul(pr[:, 128 * d:128 * (d + 1)],
                                             pr[:, 128 * d:128 * (d + 1)],
                                             tri_sb)
                    cur.append((kt, j, pr, c0))
                emit_attnv(pend)
                pend = cur
                yield
            emit_attnv(pend)
            yield

        def seq(*gens):
            for g in gens:
                yield from g

        def drive(a, b_):
            """Alternate steps of two generators until both exhausted."""
            gens = [a, b_]
            while gens:
                for g in list(gens):
                    try:
                        next(g)
                    except StopIteration:
                        gens.remove(g)

        # phase 1: batch-0 stats+QKV, then its V transposes
        for j in range(4):
            for _ in chunk_steps(j):
                pass
        for _ in vt_block(0):
            pass
        # phase 2: batch-0 attention interleaved with batch-1 stats+QKV
        drive(seq(attn_group(0, 0), attn_group(0, 1)),
              seq(chunk_steps(4), chunk_steps(5), chunk_steps(6),
                  chunk_steps(7), vt_block(1)))
        # phase 3: batch-1 attention
        for _ in seq(attn_group(1, 0), attn_group(1, 1)):
            pass
    nc.compile()
    return nc


def build_l2(use_beta2=True, use_b2=True):
    nc = bacc.Bacc("TRN2", target_bir_lowering=False, debug=False, num_devices=NCORES)
    at_d = nc.dram_tensor("at", [C, ROWS], BF16, kind="ExternalInput").ap()
    wp_d = nc.dram_tensor("wp", [C, C], BF16, kind="ExternalInput").ap()
    xr_d = nc.dram_tensor("xr", [ROWS, C], FP32, kind="ExternalInput").ap()
    w1_d = nc.dram_tensor("w1q", [128, HT, CT * 128], BF16, kind="ExternalInput").ap()
    w2_d = nc.dram_tensor("w2", [HID, C], BF16, kind="ExternalInput").ap()
    b1_d = nc.dram_tensor("b1", [HID, 1], FP32, kind="ExternalInput").ap()
    beta2_d = nc.dram_tensor("beta2", [1, C], FP32, kind="ExternalInput").ap()
    b2_d = nc.dram_tensor("b2", [1, C], FP32, kind="ExternalInput").ap()
    out_d = nc.dram_tensor("out_rows", [ROWS, C], FP32, kind="ExternalOutput").ap()

    w2r = w2_d.rearrange("(a p) m -> p a m", p=128)

    with tile.TileContext(nc) as tc, contextlib.ExitStack() as ctx:
        consts = ctx.enter_context(tc.tile_pool(name="consts", bufs=1))
        persist = ctx.enter_context(tc.tile_pool(name="persist", bufs=1))
        w1pool = ctx.enter_context(tc.tile_pool(name="w1pool", bufs=2))
        tmp = ctx.enter_context(tc.tile_pool(name="tmp", bufs=3))
        small = ctx.enter_context(tc.tile_pool(name="small", bufs=4))
        ps_p = ctx.enter_context(tc.tile_pool(name="ps_p", bufs=3, space="PSUM"))
        ps_o = ctx.enter_context(tc.tile_pool(name="ps_o", bufs=4, space="PSUM"))

        ident = consts.tile([128, 128], BF16)
        make_identity(nc, ident)
        ones_w = consts.tile([128, 128], BF16)
        nc.vector.memset(ones_w, 1.0)
        warm_ps = ps_p.tile([128, 512], FP32, tag="mm")
        for _ in range(30):
            nc.tensor.matmul(warm_ps[:, 0:128], ones_w, ones_w, start=True, stop=True)
        eps_sb = consts.tile([128, 1], FP32)
        nc.vector.memset(eps_sb, EPS)
        b1_sb = consts.tile([128, HT], FP32)
        nc.gpsimd.dma_start(out=b1_sb, in_=b1_d.rearrange("(a p) one -> p (a one)", p=128))
        if use_beta2:
            beta2_b = consts.tile([128, C], FP32)
            nc.gpsimd.dma_start(out=beta2_b, in_=beta2_d.to_broadcast((128, C)))
        if use_b2:
            b2_b = consts.tile([128, C], FP32)
            nc.gpsimd.dma_start(out=b2_b, in_=b2_d.to_broadcast((128, C)))

        # persistent outputs of the first half
        x2_sb = persist.tile([128, MT, C], FP32, tag="x2")
        h2_sb = persist.tile([128, MT, C], BF16, tag="h2")
        h2t_sb = persist.tile([128, CT, ROWS], BF16, tag="h2t")
        h1t_sb = persist.tile([128, HT, ROWS], BF16, tag="h1t")

        # early inputs in a scoped pool: freed after proj so w2 reuses the room
        with tc.tile_pool(name="early", bufs=1) as early:
            at_sb = early.tile([128, CT, ROWS], BF16, tag="at")
            nc.sync.dma_start(out=at_sb, in_=at_d.rearrange("(a p) m -> p a m", p=128))
            wp_sb = early.tile([128, CT, C], BF16, tag="wp")
            nc.sync.dma_start(out=wp_sb, in_=wp_d.rearrange("(a p) m -> p a m", p=128))
            xr_sb = early.tile([128, MT, C], FP32, tag="xr")
            nc.scalar.dma_start(out=xr_sb, in_=xr_d.rearrange("(a p) m -> p a m", p=128))

            # w1 streamed in 2MB waves (ring of 2) on the sync ring
            waves = []
            for w in range(4):
                wt = w1pool.tile([128, 8, CT * 128], BF16, tag="w1wave")
                nc.sync.dma_start(out=wt, in_=w1_d[:, 8 * w:8 * w + 8, :])
                waves.append(wt)

            # ---- proj + residual -> x2; LN2 -> h2; transpose -> h2T ----
            for m in range(MT):
                msl = slice(m * 128, (m + 1) * 128)
                pps = []
                for n in range(2):
                    pp = ps_p.tile([128, 512], FP32, tag="mm")
                    pps.append(pp)
                for ci in range(CT):
                    for n in range(2):
                        nsl = slice(n * 512, (n + 1) * 512)
                        nc.tensor.matmul(pps[n], at_sb[:, ci, msl],
                                         wp_sb[:, ci, nsl],
                                         start=(ci == 0), stop=(ci == CT - 1))
                stats_t = small.tile([128, 2, 6], FP32, tag="bnstats")
                for n in range(2):
                    nsl = slice(n * 512, (n + 1) * 512)
                    nc.vector.tensor_add(x2_sb[:, m, nsl], pps[n], xr_sb[:, m, nsl])
                    nc.vector.bn_stats(out=stats_t[:, n, :], in_=x2_sb[:, m, nsl])
                mv = small.tile([128, 2], FP32, tag="mv")
                nc.vector.bn_aggr(out=mv, in_=stats_t)
                srt = small.tile([128, 1], FP32, tag="srt")
                nc.scalar.activation(out=srt, in_=mv[:, 1:2], func=AF.Sqrt,
                                     bias=eps_sb, scale=1.0)
                rstd = small.tile([128, 1], FP32, tag="rstd2")
                nc.vector.reciprocal(rstd, srt)
                nmr = small.tile([128, 1], FP32, tag="nmr")
                nc.vector.tensor_mul(nmr, mv[:, 0:1], rstd)
                nc.vector.tensor_scalar_mul(nmr, nmr, -1.0)
                nc.scalar.activation(out=h2_sb[:, m, :], in_=x2_sb[:, m, :],
                                     func=AF.Identity, bias=nmr, scale=rstd)
                if use_beta2:
                    nc.vector.tensor_add(h2_sb[:, m, :], h2_sb[:, m, :], beta2_b)
                for ci in range(CT):
                    pt = ps_p.tile([128, 128], BF16, tag="mm")
                    nc.tensor.transpose(pt, h2_sb[:, m, ci * 128:(ci + 1) * 128],
                                        ident)
                    nc.vector.tensor_scalar_mul(h2t_sb[:, ci, msl], pt, 1.0)

        # w2 resident in SBUF (lands in the space at/wp/xr vacated)
        w2pool = ctx.enter_context(tc.tile_pool(name="w2pool", bufs=1))
        w2_sb = w2pool.tile([128, HT, C], BF16, tag="w2")
        nc.scalar.dma_start(out=w2_sb[:, 0:16, :], in_=w2r[:, 0:16, :])
        nc.scalar.dma_start(out=w2_sb[:, 16:32, :], in_=w2r[:, 16:32, :])

        # ---- FFN1: h1T[ht] = relu(W1g.T @ h2T + b1) ----
        for ht in range(HT):
            wt = waves[ht // 8]
            ph = ps_p.tile([128, 512], FP32, tag="mm")
            for ci in range(CT):
                nc.tensor.matmul(ph, wt[:, ht % 8, ci * 128:(ci + 1) * 128],
                                 h2t_sb[:, ci, :],
                                 start=(ci == 0), stop=(ci == CT - 1))
            nc.scalar.activation(out=h1t_sb[:, ht, :], in_=ph, func=AF.Relu,
                                 bias=b1_sb[:, ht:ht + 1], scale=1.0)

        # ---- FFN2 + residual (+ b2) -> out ----
        for n in range(2):
            nsl = slice(n * 512, (n + 1) * 512)
            pos = []
            for m in range(MT):
                po = ps_o.tile([128, 512], FP32, tag="pos")
                pos.append(po)
            for ht in range(HT):
                for m in range(MT):
                    nc.tensor.matmul(pos[m],
                                     h1t_sb[:, ht, m * 128:(m + 1) * 128],
                                     w2_sb[:, ht, nsl],
                                     start=(ht == 0), stop=(ht == HT - 1))
            for m in range(MT):
                ot = tmp.tile([128, 512], FP32, tag="ot")
                nc.vector.tensor_add(ot, pos[m], x2_sb[:, m, nsl])
                if use_b2:
                    nc.vector.tensor_add(ot, ot, b2_b[:, nsl])
                nc.sync.dma_start(out=out_d[m * 128:(m + 1) * 128, nsl], in_=ot)
    nc.compile()
    return nc


# ---------------- host glue ----------------

def prep_l1_inputs(inputs):
    x = np.asarray(inputs["x"], np.float32).reshape(TOK, C)
    g1 = np.asarray(inputs["g1"], np.float32)
    beta1 = np.asarray(inputs["beta1"], np.float32)
    xt = np.ascontiguousarray(x.T).astype(bf16)
    wq = (g1[:, None] * np.asarray(inputs["Wq"], np.float32)).astype(bf16)
    wk = (g1[:, None] * np.asarray(inputs["Wk"], np.float32)).astype(bf16)
    wv = (g1[:, None] * np.asarray(inputs["Wv"], np.float32)).astype(bf16)
    tri = np.triu(np.ones((128, 128), np.float32)).astype(bf16)
    in_maps = []
    for c in range(NCORES):
        csl = slice(c * 128, (c + 1) * 128)
        nws = np.stack([-wq[:, csl].astype(np.float32).sum(0),
                        -wk[:, csl].astype(np.float32).sum(0),
                        -wv[:, csl].astype(np.float32).sum(0)], axis=1)
        wb = np.stack([wq[:, csl].astype(np.float32).T @ beta1,
                       wk[:, csl].astype(np.float32).T @ beta1,
                       wv[:, csl].astype(np.float32).T @ beta1], axis=1)
        in_maps.append({
            "xt": xt,
            "wq": np.ascontiguousarray(wq[:, csl]),
            "wk": np.ascontiguousarray(wk[:, csl]),
            "wv": np.ascontiguousarray(wv[:, csl]),
            "nws": np.ascontiguousarray(nws.astype(np.float32)),
            "wb": np.ascontiguousarray(wb.astype(np.float32)),
            "tri": tri,
        })
    return in_maps


def prep_l2_inputs(inputs, attn_t):
    attn_t = np.ascontiguousarray(np.asarray(attn_t, bf16))
    x = np.asarray(inputs["x"], np.float32).reshape(TOK, C)
    g2 = np.asarray(inputs["g2"], np.float32)
    wp = np.asarray(inputs["Wp"], np.float32).astype(bf16)
    w1 = (g2[:, None] * np.asarray(inputs["W1"], np.float32)).astype(bf16)
    w1q = np.ascontiguousarray(
        w1.reshape(CT, 128, HT, 128).transpose(1, 2, 0, 3).reshape(128, HT, CT * 128))
    w2 = np.asarray(inputs["W2"], np.float32).astype(bf16)
    b1 = np.ascontiguousarray(np.asarray(inputs["b1"], np.float32).reshape(HID, 1))
    x = x + np.asarray(inputs["bp"], np.float32)[None, :]
    beta2 = np.ascontiguousarray(np.asarray(inputs["beta2"], np.float32).reshape(1, C))
    b2 = np.ascontiguousarray(np.asarray(inputs["b2"], np.float32).reshape(1, C))
    in_maps = []
    for c in range(NCORES):
        rsl = slice(c * ROWS, (c + 1) * ROWS)
        in_maps.append({
            "at": np.ascontiguousarray(attn_t[:, rsl]),
            "wp": wp,
            "xr": np.ascontiguousarray(x[rsl, :]),
            "w1q": w1q,
            "w2": w2,
            "b1": b1,
            "beta2": beta2,
            "b2": b2,
        })
    return in_maps


_CACHE = {}


def _get_programs(use_beta, use_beta2=False, use_b2=False):
    key = ("progs", bool(use_beta), bool(use_beta2), bool(use_b2))
    if key not in _CACHE:
        nc1 = build_l1(use_beta=use_beta)
        nc2 = build_l2(use_beta2=use_beta2, use_b2=use_b2)
        _CACHE[key] = (nc1, nc2)
    return _CACHE[key]


def kernel(**inputs):
    from concourse.bass_utils import run_bass_kernel_spmd

    inputs = {k: np.asarray(v) for k, v in inputs.items()}
    use_beta = bool(np.any(np.asarray(inputs["beta1"], np.float32) != 0.0))
    use_beta2 = bool(np.any(np.asarray(inputs["beta2"], np.float32) != 0.0))
    use_b2 = bool(np.any(np.asarray(inputs["b2"], np.float32) != 0.0))
    nc1, nc2 = _get_programs(use_beta, use_beta2, use_b2)
    core_ids = list(range(NCORES))

    r1 = run_bass_kernel_spmd(nc1, prep_l1_inputs(inputs), core_ids)
    attn_t = np.concatenate(
        [np.asarray(r1.results[c]["attn_out"]) for c in range(NCORES)], axis=0)

    r2 = run_bass_kernel_spmd(nc2, prep_l2_inputs(inputs, attn_t), core_ids)
    out = np.concatenate(
        [np.asarray(r2.results[c]["out_rows"]) for c in range(NCORES)], axis=0)
    return np.ascontiguousarray(out.reshape(B, T, C).astype(np.float32))
